# revision 9
# baseline (speedup 1.0000x reference)
"""Encoder-decoder LSTM seq2seq loss kernel for 8 TRN2 NeuronCores.

Strategy:
  - LSTM recurrences (encoder 48 steps, decoder 47 steps) are replicated on
    every core in gate-major layout: gates^T [2048, 64] computed as 16
    [128,64] PSUM chunks, state kept transposed (hT [128, 4*64]) so no
    per-step transposes are needed.
  - Input-side gate contributions (x @ W_ih^T + b) are batched in 8-step
    windows as full-utilization [128,128]x[128,512] matmuls (the x stream is
    known ahead of time; only the h-part is sequential).
  - The 47 decoder logit matmuls are deferred until after the recurrence
    (the loss does not feed back) and run as one big GEMM against the
    core's 4000-row vocab shard (padded to 4096), step*batch-major, so the
    softmax denominator falls out of the ACT Exp instruction's free-axis
    accumulator for free.
  - Target logits come from a host-pregathered W_out[tgt] (dot with h via
    DVE multiply + ones-matmul contraction).
  - Host combines per-core partial sum-exp + target logits into the scalar
    loss (tiny: 8 x [128,24] + [1,3008]).
"""

import sys

sys.path.insert(0, "/opt/trn_rl_repo")

import numpy as np
import ml_dtypes

BF16 = ml_dtypes.bfloat16

# Model dims (hardcoded per contract)
SRC, TGT, B, H, V = 48, 48, 64, 512, 32000
DEC = TGT - 1                  # 47 decoder steps
SB = DEC * B                   # 3008 (step*batch)
SBC = 24                       # ceil(3008/128) sb-chunks
SBP = SBC * 128                # 3072 padded
NCORES = 8
VSH = V // NCORES              # 4000 vocab rows per core
VSP = 4096                     # padded shard
WIN = 8                        # bulk x-part window (steps)
NG = 16                        # gate chunks (2048/128)
KC = 4                         # hidden chunks (512/128)

_COMPILED = None


def _build():
    import concourse.bass as bass
    import concourse.bacc as bacc
    import concourse.tile as tile
    from concourse import mybir

    f32 = mybir.dt.float32
    bf16 = mybir.dt.bfloat16
    AF = mybir.ActivationFunctionType

    nc = bacc.Bacc("TRN2", target_bir_lowering=False, debug=False,
                   num_devices=NCORES)

    def din(name, shape, dt=bf16):
        return nc.dram_tensor(name, shape, dt, kind="ExternalInput").ap()

    xt_enc = din("xt_enc", [H, SRC * B])
    xt_dec = din("xt_dec", [H, SB])
    wi_e = din("wi_e", [KC, 128, 4 * H])
    wh_e = din("wh_e", [KC, 128, 4 * H])
    wi_d = din("wi_d", [KC, 128, 4 * H])
    wh_d = din("wh_d", [KC, 128, 4 * H])
    bias_e = din("bias_e", [128, NG], f32)
    bias_d = din("bias_d", [128, NG], f32)
    mask_in = din("mask", [SRC, 128, KC * B], mybir.dt.uint8)
    wot_in = din("wot", [KC, 128, VSP])
    bout_in = din("bout", [128, VSP])
    wtgt_in = din("wtgt", [KC, 128, SB])

    out_s = nc.dram_tensor("out_s", [128, SBC], f32, kind="ExternalOutput").ap()
    out_l = nc.dram_tensor("out_l", [1, SB], f32, kind="ExternalOutput").ap()

    with tile.TileContext(nc) as tc:
        from contextlib import ExitStack
        with ExitStack() as ctx:
            # ---- pools ----
            pconst = ctx.enter_context(tc.tile_pool(name="const", bufs=1))
            pht = ctx.enter_context(tc.tile_pool(name="ht", bufs=1))
            pgx = ctx.enter_context(tc.tile_pool(name="gx", bufs=2))
            pw = ctx.enter_context(tc.tile_pool(name="w", bufs=1))
            pxt = ctx.enter_context(tc.tile_pool(name="xtw", bufs=2))
            pstate = ctx.enter_context(tc.tile_pool(name="state", bufs=3))
            pact = ctx.enter_context(tc.tile_pool(name="act", bufs=3))
            pmask = ctx.enter_context(tc.tile_pool(name="mask", bufs=2))
            psum = ctx.enter_context(
                tc.tile_pool(name="ps", bufs=2, space=bass.MemorySpace.PSUM))
            plog = ctx.enter_context(tc.tile_pool(name="log", bufs=4))

            # ---- constants ----
            # Constants are bounced through a DVE copy so downstream DVE
            # consumers dep on same-engine program order instead of adding
            # a DMA semaphore wait (ISA wait-slot limit).
            def dve_const(src_ap, shape, dt, tag):
                dma_t = pconst.tile(shape, dt, tag=f"{tag}_dma")
                nc.sync.dma_start(dma_t[:], src_ap)
                t = pconst.tile(shape, dt, tag=tag)
                nc.vector.tensor_copy(t[:], dma_t[:])
                return t

            bias_e_t = dve_const(bias_e[:], [128, NG], f32, "be")
            bias_d_t = dve_const(bias_d[:], [128, NG], f32, "bd")
            ones_t = pconst.tile([128, 1], f32)
            nc.vector.memset(ones_t[:], 1.0)

            # HT: decoder hidden states, transposed, col = k*SBP + t*64 + b
            ht = pht.tile([128, KC * SBP], bf16)
            nc.vector.memset(ht[:], 0.0)

            def load_w(dram, pool, tag, width=4 * H):
                ts = []
                dw = dram.shape[2]
                for k in range(KC):
                    t = pool.tile([128, width], bf16, tag=f"{tag}{k}")
                    nc.sync.dma_start(t[:, :dw], dram[k])
                    ts.append(t)
                return ts

            def bulk_x(xt_src, wi_t, bias_t, t0, nsteps):
                """x-part gates for steps [t0, t0+nsteps) -> gx tile.

                gx col layout: g*(WIN*B) + lt*B + b  (lt = step - t0)
                """
                w = nsteps * B
                gx = pgx.tile([128, NG * WIN * B], bf16, tag="gx")
                xtw = []
                for k in range(KC):
                    t = pxt.tile([128, WIN * B], bf16, tag=f"xt{k}")
                    nc.sync.dma_start(
                        t[:, :w], xt_src[k * 128:(k + 1) * 128,
                                         t0 * B:t0 * B + w])
                    xtw.append(t)
                for g in range(NG):
                    pb = psum.tile([128, 2048], f32, tag="ps")
                    for k in range(KC):
                        nc.tensor.matmul(pb[:, :w],
                                         wi_t[k][:, g * 128:(g + 1) * 128],
                                         xtw[k][:, :w],
                                         start=(k == 0), stop=(k == KC - 1))
                    nc.vector.tensor_scalar_add(
                        gx[:, g * WIN * B:g * WIN * B + w], pb[:, :w],
                        bias_t[:, g:g + 1])
                return gx

            def lstm_step(gx, lt, h_rhs, c_prev, wh_t, mask_t, h_out_ap):
                """One recurrence step.

                h_rhs: callable k -> AP [128, 64] (transposed h chunks)
                h_out_ap: AP [128, 4, 64] to write new h (bf16)
                returns c_new tile
                """
                pg = psum.tile([128, 2048], f32, tag="ps")
                g1024 = pg[:, :NG * B]
                for g in range(NG):
                    for k in range(KC):
                        nc.tensor.matmul(g1024[:, g * B:(g + 1) * B],
                                         wh_t[k][:, g * 128:(g + 1) * 128],
                                         h_rhs(k),
                                         start=(k == 0), stop=(k == KC - 1))
                # add x-part (+bias, already folded in)
                gx_r = gx[:].rearrange("p (g s) -> p g s", g=NG)
                nc.vector.tensor_add(
                    g1024.rearrange("p (g s) -> p g s", g=NG),
                    g1024.rearrange("p (g s) -> p g s", g=NG),
                    gx_r[:, :, lt * B:(lt + 1) * B])
                # activations: chunks [i(0:4) f(4:8) o(8:12) g(12:16)]
                sig = pact.tile([128, 768], f32, tag="sig")
                nc.scalar.activation(sig[:], g1024[:, 0:768], AF.Sigmoid)
                tng = pact.tile([128, 256], f32, tag="tng")
                nc.scalar.activation(tng[:], g1024[:, 768:1024], AF.Tanh)
                # c2 = sig_f*c + sig_i*tanh_g
                t1 = pact.tile([128, 256], f32, tag="t1")
                nc.vector.tensor_mul(t1[:], sig[:, 256:512], c_prev[:])
                t2 = pact.tile([128, 256], f32, tag="t2")
                nc.vector.tensor_mul(t2[:], sig[:, 0:256], tng[:])
                c_new = pstate.tile([128, 256], f32, tag="c")
                nc.vector.tensor_add(c_new[:], t1[:], t2[:])
                tnc = pact.tile([128, 256], f32, tag="tnc")
                nc.scalar.activation(tnc[:], c_new[:], AF.Tanh)
                nc.vector.tensor_mul(
                    h_out_ap,
                    sig[:, 512:768].rearrange("p (k s) -> p k s", k=KC),
                    tnc[:].rearrange("p (k s) -> p k s", k=KC))
                return c_new

            # ================= encoder =================
            we_i = load_w(wi_e, pw, "wie")
            we_h = load_w(wh_e, pw, "whe")

            h_prev = pstate.tile([128, KC * B], bf16, tag="h")
            nc.vector.memset(h_prev[:], 0.0)
            c_prev = pstate.tile([128, 256], f32, tag="c")
            nc.vector.memset(c_prev[:], 0.0)

            gx = None
            for t in range(SRC):
                if t % WIN == 0:
                    gx = bulk_x(xt_enc, we_i, bias_e_t, t, min(WIN, SRC - t))
                h_new = pstate.tile([128, KC * B], bf16, tag="h")
                hp = h_prev
                c_new = lstm_step(
                    gx, t % WIN,
                    lambda k, hp=hp: hp[:, k * B:(k + 1) * B],
                    c_prev, we_h,
                    None,
                    h_new[:].rearrange("p (k s) -> p k s", k=KC))
                # padding mask: restore previous state where tok == 0
                mk = pmask.tile([128, KC * B], mybir.dt.uint8, tag="mk")
                nc.sync.dma_start(mk[:], mask_in[t])
                nc.vector.copy_predicated(h_new[:], mk[:], h_prev[:])
                nc.vector.copy_predicated(c_new[:], mk[:], c_prev[:])
                h_prev, c_prev = h_new, c_new

            # ================= decoder recurrence =================
            wd_i = load_w(wi_d, pw, "wie")   # reuse encoder slots
            wd_h = load_w(wh_d, pw, "whe")

            for t in range(DEC):
                if t % WIN == 0:
                    gx = bulk_x(xt_dec, wd_i, bias_d_t, t, min(WIN, DEC - t))
                if t == 0:
                    rhs = lambda k, hp=h_prev: hp[:, k * B:(k + 1) * B]
                else:
                    rhs = lambda k, tp=t - 1: ht[:, k * SBP + tp * B:
                                                 k * SBP + (tp + 1) * B]
                c_prev = lstm_step(
                    gx, t % WIN, rhs, c_prev, wd_h, None,
                    ht[:].rearrange("p (k s) -> p k s", k=KC)
                    [:, :, t * B:(t + 1) * B])

            # ================= target logits =================
            wtg = load_w(wtgt_in, pw, "big", width=VSP)
            l_sb = pconst.tile([1, SB], f32)
            for nt in range(6):
                wdt = min(512, SB - nt * 512)
                pt = psum.tile([128, 2048], f32, tag="ps")
                for k in range(KC):
                    prod = plog.tile([128, 512], f32, tag="prod")
                    nc.vector.tensor_mul(
                        prod[:, :wdt],
                        ht[:, k * SBP + nt * 512:k * SBP + nt * 512 + wdt],
                        wtg[k][:, nt * 512:nt * 512 + wdt])
                    nc.tensor.matmul(pt[0:1, :wdt], ones_t[:], prod[:, :wdt],
                                     start=(k == 0), stop=(k == KC - 1))
                nc.scalar.copy(l_sb[:, nt * 512:nt * 512 + wdt], pt[0:1, :wdt])
            nc.sync.dma_start(out_l[:], l_sb[:])

            # ================= vocab-shard logits + sum-exp =================
            wot = load_w(wot_in, pw, "big", width=VSP)
            bout = dve_const(bout_in[:], [128, VSP], bf16, "bo")
            s_all = pconst.tile([128, SBC], f32)

            for sb in range(SBC):
                sh = []
                for half in range(2):
                    pl = psum.tile([128, 2048], f32, tag="ps")
                    for v4 in range(4):
                        col = half * 2048 + v4 * 512
                        for k in range(KC):
                            nc.tensor.matmul(
                                pl[:, v4 * 512:(v4 + 1) * 512],
                                ht[:, k * SBP + sb * 128:
                                   k * SBP + (sb + 1) * 128],
                                wot[k][:, col:col + 512],
                                start=(k == 0), stop=(k == KC - 1))
                    nc.vector.tensor_add(pl[:], pl[:],
                                         bout[:, half * 2048:(half + 1) * 2048])
                    sh_t = plog.tile([128, 1], f32, tag="sh")
                    nc.scalar.activation(pl[:], pl[:], AF.Exp,
                                         accum_out=sh_t[:])
                    sh.append(sh_t)
                nc.vector.tensor_add(s_all[:, sb:sb + 1], sh[0][:], sh[1][:])
            nc.sync.dma_start(out_s[:], s_all[:])

    nc.compile()
    return nc


def _prep(inputs):
    """Host-side data prep. Returns per-core in_maps + host combine data."""
    il = np.asarray(inputs["input_lines"])
    tl = np.asarray(inputs["target_lines"])
    f = lambda k: np.asarray(inputs[k], np.float32)
    emb_in, emb_tgt = f("emb_in").copy(), f("emb_tgt").copy()
    emb_in[0] = 0.0
    emb_tgt[0] = 0.0
    W_out, b_out = f("W_out"), f("b_out")

    perm = np.concatenate([np.arange(0, 512), np.arange(512, 1024),
                           np.arange(1536, 2048), np.arange(1024, 1536)])

    def wt(w):  # [2048,512] -> [4,128,2048] bf16 (transposed, gate-permuted)
        return np.ascontiguousarray(
            w[perm].T.reshape(KC, 128, 4 * H)).astype(BF16)

    def bias(bi, bh):  # -> [128, 16] f32
        return np.ascontiguousarray(
            (bi + bh)[perm].reshape(NG, 128).T).astype(np.float32)

    x_enc = emb_in[il.reshape(-1)]                       # [3072, 512]
    xt_enc = np.ascontiguousarray(x_enc.T).astype(BF16)  # [512, 3072]
    tgt_in = tl[:DEC].reshape(-1)
    x_dec = emb_tgt[tgt_in]
    xt_dec = np.ascontiguousarray(x_dec.T).astype(BF16)  # [512, 3008]

    m = (il == 0).astype(np.float32)                     # [48, 64]
    mask = np.ascontiguousarray(np.broadcast_to(
        m[:, None, None, :], (SRC, 128, KC, B)).reshape(
            SRC, 128, KC * B)).astype(np.uint8)

    tgt_next = tl[1:TGT].reshape(-1)                     # [3008]
    wtgt = np.ascontiguousarray(
        W_out[tgt_next].T.reshape(KC, 128, SB)).astype(BF16)
    b_tgt = b_out[tgt_next].astype(np.float64)

    common = dict(
        xt_enc=xt_enc, xt_dec=xt_dec,
        wi_e=wt(f("W_ih_e")), wh_e=wt(f("W_hh_e")),
        wi_d=wt(f("W_ih_d")), wh_d=wt(f("W_hh_d")),
        bias_e=bias(f("b_ih_e"), f("b_hh_e")),
        bias_d=bias(f("b_ih_d"), f("b_hh_d")),
        mask=mask, wtgt=wtgt,
    )
    in_maps = []
    for c in range(NCORES):
        ws = np.zeros((VSP, H), np.float32)
        ws[:VSH] = W_out[c * VSH:(c + 1) * VSH]
        bs = np.full(VSP, -88.0, np.float32)
        bs[:VSH] = b_out[c * VSH:(c + 1) * VSH]
        in_maps.append(dict(
            common,
            wot=np.ascontiguousarray(ws.T.reshape(KC, 128, VSP)).astype(BF16),
            bout=np.ascontiguousarray(
                np.broadcast_to(bs, (128, VSP))).astype(BF16),
        ))
    return in_maps, b_tgt


def _combine(results, b_tgt):
    s = np.zeros(SBP, np.float64)
    for r in results:
        s += np.asarray(r["out_s"], np.float64).T.reshape(-1)
    s = s[:SB]
    lse = np.log(s)
    l_tgt = np.asarray(results[0]["out_l"], np.float64).reshape(-1) + b_tgt
    return np.float32((lse - l_tgt).sum() / B)


def kernel(**inputs):
    global _COMPILED
    from concourse.bass_utils import run_bass_kernel_spmd
    in_maps, b_tgt = _prep(inputs)
    if _COMPILED is None:
        _COMPILED = _build()
    res = run_bass_kernel_spmd(_COMPILED, in_maps, list(range(NCORES)))
    return _combine(res.results, b_tgt)


if __name__ == "__main__":
    import reference
    inp = reference.setup_inputs()
    expected = np.asarray(reference.reference(**inp))
    actual = kernel(**{k: np.asarray(v) for k, v in inp.items()})
    err = abs(actual - expected) / max(abs(expected), 1e-9)
    print(f"expected={expected} actual={actual} rel_err={err:.3e}")


# revision 16
# speedup vs baseline: 1.7109x; 1.7109x over previous
"""Encoder-decoder LSTM seq2seq loss kernel for 8 TRN2 NeuronCores.

Strategy:
  - LSTM recurrences (encoder 48 steps, decoder 47 steps) are replicated on
    every core in gate-major layout: gates^T [2048, 64] computed as 16
    [128,64] PSUM chunks, state kept transposed (hT [128, 4*64]) so no
    per-step transposes are needed.
  - Input-side gate contributions (x @ W_ih^T + b) are batched in 8-step
    windows as full-utilization [128,128]x[128,512] matmuls, and the
    window matmuls are interleaved between recurrence steps so they fill
    PE idle gaps. The per-step x-injection into the gates PSUM is done by
    the PE itself (identity matmul, exact for 1.0*bf16) so the critical
    h-chain has no extra DVE hop.
  - Gates PSUM is split into three tiles [i|f], [g], [o] with the g
    chunks issued first so the c-path (tanh g, c update, tanh c) runs
    under the remaining matmuls; the o chunks are issued last so the
    h tail is just sigmoid(o) * tanh(c).
  - The 47 decoder logit matmuls are deferred until after the recurrence
    (the loss does not feed back) and run as one big GEMM against the
    core's 4000-row vocab shard (padded to 4096), step*batch-major, so the
    softmax denominator falls out of the ACT Exp instruction's free-axis
    accumulator for free.
  - Target logits come from a host-pregathered W_out[tgt] (dot with h via
    DVE multiply + ones-matmul contraction).
  - Host combines per-core partial sum-exp + target logits into the scalar
    loss (tiny: 8 x [128,24] + [1,3008]).
"""

import sys

sys.path.insert(0, "/opt/trn_rl_repo")

import numpy as np
import ml_dtypes

BF16 = ml_dtypes.bfloat16

# Model dims (hardcoded per contract)
SRC, TGT, B, H, V = 48, 48, 64, 512, 32000
DEC = TGT - 1                  # 47 decoder steps
SB = DEC * B                   # 3008 (step*batch)
SBC = 24                       # ceil(3008/128) sb-chunks
SBP = SBC * 128                # 3072 padded
NCORES = 8
VSH = V // NCORES              # 4000 vocab rows per core
VSP = 4096                     # padded shard
WIN = 8                        # bulk x-part window (steps)
NG = 16                        # gate chunks (2048/128)
KC = 4                         # hidden chunks (512/128)

# gate-chunk indices in the permuted [i f o g] weight layout
I_CH = list(range(0, 4))
F_CH = list(range(4, 8))
O_CH = list(range(8, 12))
G_CH = list(range(12, 16))

_COMPILED = None


def _build():
    import concourse.bass as bass
    import concourse.bacc as bacc
    import concourse.tile as tile
    from concourse import mybir

    f32 = mybir.dt.float32
    bf16 = mybir.dt.bfloat16
    AF = mybir.ActivationFunctionType

    nc = bacc.Bacc("TRN2", target_bir_lowering=False, debug=False,
                   num_devices=NCORES)

    def din(name, shape, dt=bf16):
        return nc.dram_tensor(name, shape, dt, kind="ExternalInput").ap()

    xt_enc = din("xt_enc", [H, SRC * B])
    xt_dec = din("xt_dec", [H, SB])
    wi_e = din("wi_e", [KC, 128, 4 * H])
    wh_e = din("wh_e", [KC, 128, 4 * H])
    wi_d = din("wi_d", [KC, 128, 4 * H])
    wh_d = din("wh_d", [KC, 128, 4 * H])
    bias_e = din("bias_e", [128, NG], f32)
    bias_d = din("bias_d", [128, NG], f32)
    mask_in = din("mask", [SRC, 128, KC * B], mybir.dt.uint8)
    ident_in = din("ident", [128, 128])
    wot_in = din("wot", [KC, 128, VSP])
    bout_in = din("bout", [128, VSP])
    wtgt_in = din("wtgt", [KC, 128, SB])

    out_s = nc.dram_tensor("out_s", [128, SBC], f32, kind="ExternalOutput").ap()
    out_l = nc.dram_tensor("out_l", [1, SB], f32, kind="ExternalOutput").ap()

    with tile.TileContext(nc) as tc:
        from contextlib import ExitStack
        with ExitStack() as ctx:
            # ---- pools ----
            pconst = ctx.enter_context(tc.tile_pool(name="const", bufs=1))
            pht = ctx.enter_context(tc.tile_pool(name="ht", bufs=1))
            pgx = ctx.enter_context(tc.tile_pool(name="gx", bufs=2))
            pw = ctx.enter_context(tc.tile_pool(name="w", bufs=1))
            pxt = ctx.enter_context(tc.tile_pool(name="xtw", bufs=2))
            pstate = ctx.enter_context(tc.tile_pool(name="state", bufs=3))
            pact = ctx.enter_context(tc.tile_pool(name="act", bufs=2))
            pmask = ctx.enter_context(tc.tile_pool(name="mask", bufs=2))
            # (log pool is small: prod + sh tiles in logits phase)
            plog = ctx.enter_context(tc.tile_pool(name="log", bufs=2))

            # ---- constants ----
            def dve_const(src_ap, shape, dt, tag):
                dma_t = pconst.tile(shape, dt, tag=f"{tag}_dma")
                nc.sync.dma_start(dma_t[:], src_ap)
                t = pconst.tile(shape, dt, tag=tag)
                nc.vector.tensor_copy(t[:], dma_t[:])
                return t

            bias_e_t = dve_const(bias_e[:], [128, NG], f32, "be")
            bias_d_t = dve_const(bias_d[:], [128, NG], f32, "bd")
            ones_t = pconst.tile([128, 1], f32)
            nc.vector.memset(ones_t[:], 1.0)
            ident = pconst.tile([128, 128], bf16)
            nc.sync.dma_start(ident[:], ident_in[:])

            # HT: decoder hidden states, transposed, col = k*SBP + t*64 + b
            ht = pht.tile([128, KC * SBP], bf16)
            nc.vector.memset(ht[:], 0.0)

            def load_w(dram, pool, tag, width=4 * H):
                ts = []
                dw = dram.shape[2]
                for k in range(KC):
                    t = pool.tile([128, width], bf16, tag=f"{tag}{k}")
                    nc.sync.dma_start(t[:, :dw], dram[k])
                    ts.append(t)
                return ts

            we_i = load_w(wi_e, pw, "wie")
            we_h = load_w(wh_e, pw, "whe")
            wd_i = load_w(wi_d, pw, "wid")
            wd_h = load_w(wh_d, pw, "whd")

            # ============ unified 95-step recurrence ============
            with (
                tc.tile_pool(name="psA", bufs=3, space=bass.MemorySpace.PSUM)
                    as psA,
                tc.tile_pool(name="psB", bufs=2, space=bass.MemorySpace.PSUM)
                    as psB,
                tc.tile_pool(name="psC", bufs=2, space=bass.MemorySpace.PSUM)
                    as psC,
            ):
                def bulk_pieces(xt_src, wi_t, bias_t, t0, nsteps):
                    """Yield closures: piece 0 = DMA + gx alloc, one piece
                    per gate chunk (4 MMs + bias copy to gx), then a
                    sentinel returning the gx tile."""
                    w = nsteps * B
                    state = {}

                    def p_dma():
                        state["gx"] = pgx.tile([128, NG * WIN * B], bf16,
                                               tag="gx", name="gxw")
                        xtw = []
                        for k in range(KC):
                            t = pxt.tile([128, WIN * B], bf16, tag=f"xt{k}")
                            nc.sync.dma_start(
                                t[:, :w], xt_src[k * 128:(k + 1) * 128,
                                                 t0 * B:t0 * B + w])
                            xtw.append(t)
                        state["xtw"] = xtw
                    yield p_dma

                    def mk_chunk(g):
                        def p_chunk():
                            pb = psA.tile([128, 512], f32, tag="psA")
                            for k in range(KC):
                                nc.tensor.matmul(
                                    pb[:, :w],
                                    wi_t[k][:, g * 128:(g + 1) * 128],
                                    state["xtw"][k][:, :w],
                                    start=(k == 0), stop=(k == KC - 1))
                            nc.vector.tensor_scalar_add(
                                state["gx"][:, g * WIN * B:g * WIN * B + w],
                                pb[:, :w], bias_t[:, g:g + 1])
                        return p_chunk
                    for g in range(NG):
                        yield mk_chunk(g)
                    yield lambda: state["gx"]

                def lstm_step(gx, lt, h_rhs, c_prev, wh_t, h_out_ap):
                    """One step. g chunks issue first (c-path overlaps the
                    i/f/o matmuls), o last (short h tail)."""
                    pA = psA.tile([128, 512], f32, tag="psA")  # i|f
                    pB = psB.tile([128, 256], f32, tag="psB")  # g
                    pC = psC.tile([128, 256], f32, tag="psC")  # o

                    def dst(c):
                        if c in G_CH:
                            return pB[:, (c - 12) * B:(c - 11) * B]
                        if c in O_CH:
                            return pC[:, (c - 8) * B:(c - 7) * B]
                        return pA[:, c * B:(c + 1) * B]
                    order = G_CH + I_CH + F_CH + O_CH
                    gx_r = gx[:].rearrange("p (g s) -> p g s", g=NG)
                    # x-part injection: identity stationary, one wide
                    # matmul per PSUM tile (one accumulation group each)
                    nc.tensor.matmul(
                        pB[:].rearrange("p (g s) -> p g s", g=4),
                        ident[:], gx_r[:, 12:16, lt * B:(lt + 1) * B],
                        start=True, stop=False)
                    nc.tensor.matmul(
                        pA[:].rearrange("p (g s) -> p g s", g=8),
                        ident[:], gx_r[:, 0:8, lt * B:(lt + 1) * B],
                        start=True, stop=False)
                    nc.tensor.matmul(
                        pC[:].rearrange("p (g s) -> p g s", g=4),
                        ident[:], gx_r[:, 8:12, lt * B:(lt + 1) * B],
                        start=True, stop=False)
                    # h-part; last matmul into each tile carries stop
                    for c in order:
                        for k in range(KC):
                            last = (k == KC - 1) and c in (15, 7, 11)
                            nc.tensor.matmul(
                                dst(c),
                                wh_t[k][:, c * 128:(c + 1) * 128],
                                h_rhs(k),
                                start=False, stop=last)
                    # ACT: g's tanh first (its matmuls finished first)
                    tng = pact.tile([128, 256], f32, tag="tng")
                    nc.scalar.activation(tng[:], pB[:], AF.Tanh)
                    sig = pact.tile([128, 512], f32, tag="sig")
                    nc.scalar.activation(sig[:], pA[:], AF.Sigmoid)
                    sgo = pact.tile([128, 256], f32, tag="sgo")
                    nc.scalar.activation(sgo[:], pC[:], AF.Sigmoid)
                    # c2 = sig_f*c + sig_i*tanh_g
                    t2 = pact.tile([128, 256], f32, tag="t2")
                    nc.vector.tensor_mul(t2[:], sig[:, 0:256], tng[:])
                    t1 = pact.tile([128, 256], f32, tag="t1")
                    nc.vector.tensor_mul(t1[:], sig[:, 256:512], c_prev[:])
                    c_new = pstate.tile([128, 256], f32, tag="c")
                    nc.vector.tensor_add(c_new[:], t1[:], t2[:])
                    tnc = pact.tile([128, 256], f32, tag="tnc")
                    nc.scalar.activation(tnc[:], c_new[:], AF.Tanh)
                    nc.vector.tensor_mul(
                        h_out_ap,
                        sgo[:].rearrange("p (k s) -> p k s", k=KC),
                        tnc[:].rearrange("p (k s) -> p k s", k=KC))
                    return c_new

                h_prev = pstate.tile([128, KC * B], bf16, tag="h")
                nc.vector.memset(h_prev[:], 0.0)
                c_prev = pstate.tile([128, 256], f32, tag="c")
                nc.vector.memset(c_prev[:], 0.0)

                win_list = (
                    [(xt_enc, we_i, bias_e_t, t0, min(WIN, SRC - t0))
                     for t0 in range(0, SRC, WIN)] +
                    [(xt_dec, wd_i, bias_d_t, t0, min(WIN, DEC - t0))
                     for t0 in range(0, DEC, WIN)])

                gx = None
                for p in bulk_pieces(*win_list[0]):   # prologue window
                    r = p()
                    gx = r if r is not None else gx
                next_idx = 1
                next_gen = bulk_pieces(*win_list[next_idx])
                gx_next = None

                step_no = 0
                for phase, nsteps in (("enc", SRC), ("dec", DEC)):
                    wh_t = we_h if phase == "enc" else wd_h
                    for t in range(nsteps):
                        if t % WIN == 0 and step_no > 0:
                            # window switch: finish pending bulk, swap gx
                            while next_gen is not None:
                                try:
                                    p = next(next_gen)
                                except StopIteration:
                                    next_gen = None
                                    break
                                r = p()
                                gx_next = r if r is not None else gx_next
                            gx, gx_next = gx_next, None
                            next_idx += 1
                            if next_idx < len(win_list):
                                next_gen = bulk_pieces(*win_list[next_idx])
                        if phase == "enc" or t == 0:
                            hp = h_prev
                            rhs = (lambda k, hp=hp:
                                   hp[:, k * B:(k + 1) * B])
                        else:
                            rhs = (lambda k, tp=t - 1:
                                   ht[:, k * SBP + tp * B:
                                      k * SBP + (tp + 1) * B])
                        if phase == "enc":
                            h_new = pstate.tile([128, KC * B], bf16, tag="h")
                            out_ap = h_new[:].rearrange(
                                "p (k s) -> p k s", k=KC)
                        else:
                            out_ap = ht[:].rearrange(
                                "p (k s) -> p k s",
                                k=KC)[:, :, t * B:(t + 1) * B]
                        c_new = lstm_step(gx, t % WIN, rhs, c_prev, wh_t,
                                          out_ap)
                        if phase == "enc":
                            mk = pmask.tile([128, KC * B], mybir.dt.uint8,
                                            tag="mk")
                            nc.sync.dma_start(mk[:], mask_in[t])
                            nc.vector.copy_predicated(h_new[:], mk[:],
                                                      h_prev[:])
                            nc.vector.copy_predicated(c_new[:], mk[:],
                                                      c_prev[:])
                            h_prev = h_new
                        c_prev = c_new
                        step_no += 1
                        # interleave next window's bulk (3 pieces/step)
                        if next_gen is not None:
                            for _ in range(3):
                                try:
                                    p = next(next_gen)
                                except StopIteration:
                                    next_gen = None
                                    break
                                r = p()
                                gx_next = r if r is not None else gx_next

            # ================= target logits =================
            with tc.tile_pool(name="psL", bufs=2,
                              space=bass.MemorySpace.PSUM) as psL:
                def load_w2(dram, nm):
                    # two [128, 2*VSP] tiles in the (now idle) gx slots
                    dw = dram.shape[2]
                    ts = []
                    for half in range(2):
                        t = pgx.tile([128, 2 * VSP], bf16, tag="gx",
                                     name=f"{nm}{half}")
                        for j in range(2):
                            nc.sync.dma_start(
                                t[:, j * VSP:j * VSP + dw], dram[half * 2 + j])
                        ts.append(t)
                    return lambda k: ts[k // 2][:, (k % 2) * VSP:
                                                (k % 2 + 1) * VSP]

                wtg_s = load_w2(wtgt_in, "wtg")
                l_sb = pconst.tile([1, SB], f32)
                for nt in range(6):
                    wdt = min(512, SB - nt * 512)
                    pt = psL.tile([128, 2048], f32, tag="psL")
                    for k in range(KC):
                        prod = plog.tile([128, 512], f32, tag="prod")
                        nc.vector.tensor_mul(
                            prod[:, :wdt],
                            ht[:, k * SBP + nt * 512:k * SBP + nt * 512 + wdt],
                            wtg_s(k)[:, nt * 512:nt * 512 + wdt])
                        nc.tensor.matmul(pt[0:1, :wdt], ones_t[:],
                                         prod[:, :wdt],
                                         start=(k == 0), stop=(k == KC - 1))
                    nc.scalar.copy(l_sb[:, nt * 512:nt * 512 + wdt],
                                   pt[0:1, :wdt])
                nc.sync.dma_start(out_l[:], l_sb[:])

                # ============ vocab-shard logits + sum-exp ============
                wot_s = load_w2(wot_in, "wot")
                bout = pconst.tile([128, VSP], bf16)
                nc.sync.dma_start(bout[:], bout_in[:])
                s_all = pconst.tile([128, SBC], f32)

                for sb in range(SBC):
                    sh = []
                    for half in range(2):
                        pl = psL.tile([128, 2048], f32, tag="psL")
                        for v4 in range(4):
                            col = half * 2048 + v4 * 512
                            for k in range(KC):
                                nc.tensor.matmul(
                                    pl[:, v4 * 512:(v4 + 1) * 512],
                                    ht[:, k * SBP + sb * 128:
                                       k * SBP + (sb + 1) * 128],
                                    wot_s(k)[:, col:col + 512],
                                    start=(k == 0), stop=(k == KC - 1))
                        nc.vector.tensor_add(
                            pl[:], pl[:],
                            bout[:, half * 2048:(half + 1) * 2048])
                        sh_t = plog.tile([128, 1], f32, tag="sh")
                        nc.scalar.activation(pl[:], pl[:], AF.Exp,
                                             accum_out=sh_t[:])
                        sh.append(sh_t)
                    nc.vector.tensor_add(s_all[:, sb:sb + 1],
                                         sh[0][:], sh[1][:])
                nc.sync.dma_start(out_s[:], s_all[:])

    nc.compile()
    return nc


def _prep(inputs):
    """Host-side data prep. Returns per-core in_maps + host combine data."""
    il = np.asarray(inputs["input_lines"])
    tl = np.asarray(inputs["target_lines"])
    f = lambda k: np.asarray(inputs[k], np.float32)
    emb_in, emb_tgt = f("emb_in").copy(), f("emb_tgt").copy()
    emb_in[0] = 0.0
    emb_tgt[0] = 0.0
    W_out, b_out = f("W_out"), f("b_out")

    perm = np.concatenate([np.arange(0, 512), np.arange(512, 1024),
                           np.arange(1536, 2048), np.arange(1024, 1536)])

    def wt(w):  # [2048,512] -> [4,128,2048] bf16 (transposed, gate-permuted)
        return np.ascontiguousarray(
            w[perm].T.reshape(KC, 128, 4 * H)).astype(BF16)

    def bias(bi, bh):  # -> [128, 16] f32
        return np.ascontiguousarray(
            (bi + bh)[perm].reshape(NG, 128).T).astype(np.float32)

    x_enc = emb_in[il.reshape(-1)]                       # [3072, 512]
    xt_enc = np.ascontiguousarray(x_enc.T).astype(BF16)  # [512, 3072]
    tgt_in = tl[:DEC].reshape(-1)
    x_dec = emb_tgt[tgt_in]
    xt_dec = np.ascontiguousarray(x_dec.T).astype(BF16)  # [512, 3008]

    m = (il == 0).astype(np.uint8)                       # [48, 64]
    mask = np.ascontiguousarray(np.broadcast_to(
        m[:, None, None, :], (SRC, 128, KC, B)).reshape(
            SRC, 128, KC * B)).astype(np.uint8)

    tgt_next = tl[1:TGT].reshape(-1)                     # [3008]
    wtgt = np.ascontiguousarray(
        W_out[tgt_next].T.reshape(KC, 128, SB)).astype(BF16)
    b_tgt = b_out[tgt_next].astype(np.float64)

    common = dict(
        xt_enc=xt_enc, xt_dec=xt_dec,
        wi_e=wt(f("W_ih_e")), wh_e=wt(f("W_hh_e")),
        wi_d=wt(f("W_ih_d")), wh_d=wt(f("W_hh_d")),
        bias_e=bias(f("b_ih_e"), f("b_hh_e")),
        bias_d=bias(f("b_ih_d"), f("b_hh_d")),
        mask=mask, wtgt=wtgt,
        ident=np.eye(128, dtype=BF16),
    )
    in_maps = []
    for c in range(NCORES):
        ws = np.zeros((VSP, H), np.float32)
        ws[:VSH] = W_out[c * VSH:(c + 1) * VSH]
        bs = np.full(VSP, -88.0, np.float32)
        bs[:VSH] = b_out[c * VSH:(c + 1) * VSH]
        in_maps.append(dict(
            common,
            wot=np.ascontiguousarray(ws.T.reshape(KC, 128, VSP)).astype(BF16),
            bout=np.ascontiguousarray(
                np.broadcast_to(bs, (128, VSP))).astype(BF16),
        ))
    return in_maps, b_tgt


def _combine(results, b_tgt):
    s = np.zeros(SBP, np.float64)
    for r in results:
        s += np.asarray(r["out_s"], np.float64).T.reshape(-1)
    s = s[:SB]
    lse = np.log(s)
    l_tgt = np.asarray(results[0]["out_l"], np.float64).reshape(-1) + b_tgt
    return np.float32((lse - l_tgt).sum() / B)


def kernel(**inputs):
    global _COMPILED
    from concourse.bass_utils import run_bass_kernel_spmd
    in_maps, b_tgt = _prep(inputs)
    if _COMPILED is None:
        _COMPILED = _build()
    res = run_bass_kernel_spmd(_COMPILED, in_maps, list(range(NCORES)))
    return _combine(res.results, b_tgt)


if __name__ == "__main__":
    import reference
    inp = reference.setup_inputs()
    expected = np.asarray(reference.reference(**inp))
    actual = kernel(**{k: np.asarray(v) for k, v in inp.items()})
    err = abs(actual - expected) / max(abs(expected), 1e-9)
    print(f"expected={expected} actual={actual} rel_err={err:.3e}")


# revision 18
# speedup vs baseline: 1.7186x; 1.0045x over previous
"""Encoder-decoder LSTM seq2seq loss kernel for 8 TRN2 NeuronCores.

Strategy:
  - LSTM recurrences (encoder 48 steps, decoder 47 steps) are replicated on
    every core in gate-major layout: gates^T [2048, 64] computed as 16
    [128,64] PSUM chunks, state kept transposed (hT [128, 4*64]) so no
    per-step transposes are needed.
  - Input-side gate contributions (x @ W_ih^T + b) are batched in 8-step
    windows as full-utilization [128,128]x[128,512] matmuls, and the
    window matmuls are interleaved between recurrence steps so they fill
    PE idle gaps. The per-step x-injection into the gates PSUM is done by
    the PE itself (identity matmul, exact for 1.0*bf16) so the critical
    h-chain has no extra DVE hop.
  - Gates PSUM is split into three tiles [i|f], [g], [o] with the g
    chunks issued first so the c-path (tanh g, c update, tanh c) runs
    under the remaining matmuls; the o chunks are issued last so the
    h tail is just sigmoid(o) * tanh(c).
  - The 47 decoder logit matmuls are deferred until after the recurrence
    (the loss does not feed back) and run as one big GEMM against the
    core's 4000-row vocab shard (padded to 4096), step*batch-major, so the
    softmax denominator falls out of the ACT Exp instruction's free-axis
    accumulator for free.
  - Target logits come from a host-pregathered W_out[tgt] (dot with h via
    DVE multiply + ones-matmul contraction).
  - Host combines per-core partial sum-exp + target logits into the scalar
    loss (tiny: 8 x [128,24] + [1,3008]).
"""

import sys

sys.path.insert(0, "/opt/trn_rl_repo")

import numpy as np
import ml_dtypes

BF16 = ml_dtypes.bfloat16

# Model dims (hardcoded per contract)
SRC, TGT, B, H, V = 48, 48, 64, 512, 32000
DEC = TGT - 1                  # 47 decoder steps
SB = DEC * B                   # 3008 (step*batch)
SBC = 24                       # ceil(3008/128) sb-chunks
SBP = SBC * 128                # 3072 padded
NCORES = 8
VSH = V // NCORES              # 4000 vocab rows per core
VSP = 4096                     # padded shard
WIN = 8                        # bulk x-part window (steps)
NG = 16                        # gate chunks (2048/128)
KC = 4                         # hidden chunks (512/128)

# gate-chunk indices in the permuted [i f o g] weight layout
I_CH = list(range(0, 4))
F_CH = list(range(4, 8))
O_CH = list(range(8, 12))
G_CH = list(range(12, 16))

_COMPILED = None


def _build():
    import concourse.bass as bass
    import concourse.bacc as bacc
    import concourse.tile as tile
    from concourse import mybir

    f32 = mybir.dt.float32
    bf16 = mybir.dt.bfloat16
    AF = mybir.ActivationFunctionType

    nc = bacc.Bacc("TRN2", target_bir_lowering=False, debug=False,
                   num_devices=NCORES)

    def din(name, shape, dt=bf16):
        return nc.dram_tensor(name, shape, dt, kind="ExternalInput").ap()

    xt_enc = din("xt_enc", [H, SRC * B])
    xt_dec = din("xt_dec", [H, SB])
    wi_e = din("wi_e", [KC, 128, 4 * H])
    wh_e = din("wh_e", [KC, 128, 4 * H])
    wi_d = din("wi_d", [KC, 128, 4 * H])
    wh_d = din("wh_d", [KC, 128, 4 * H])
    bias_e = din("bias_e", [128, NG], f32)
    bias_d = din("bias_d", [128, NG], f32)
    mask_in = din("mask", [SRC, 128, KC * B], mybir.dt.uint8)
    ident_in = din("ident", [128, 128])
    wot_in = din("wot", [KC, 128, VSP])
    bout_in = din("bout", [128, VSP])
    wtgt_in = din("wtgt", [KC, 128, SB])

    out_s = nc.dram_tensor("out_s", [128, SBC], f32, kind="ExternalOutput").ap()
    out_l = nc.dram_tensor("out_l", [1, SB], f32, kind="ExternalOutput").ap()

    with tile.TileContext(nc) as tc:
        from contextlib import ExitStack
        with ExitStack() as ctx:
            # ---- pools ----
            pconst = ctx.enter_context(tc.tile_pool(name="const", bufs=1))
            pht = ctx.enter_context(tc.tile_pool(name="ht", bufs=1))
            pgx = ctx.enter_context(tc.tile_pool(name="gx", bufs=2))
            pw = ctx.enter_context(tc.tile_pool(name="w", bufs=1))
            pxt = ctx.enter_context(tc.tile_pool(name="xtw", bufs=2))
            pstate = ctx.enter_context(tc.tile_pool(name="state", bufs=3))
            pact = ctx.enter_context(tc.tile_pool(name="act", bufs=2))
            pmask = ctx.enter_context(tc.tile_pool(name="mask", bufs=2))
            # (log pool is small: prod + sh tiles in logits phase)
            plog = ctx.enter_context(tc.tile_pool(name="log", bufs=2))

            # ---- constants ----
            def dve_const(src_ap, shape, dt, tag):
                dma_t = pconst.tile(shape, dt, tag=f"{tag}_dma")
                nc.sync.dma_start(dma_t[:], src_ap)
                t = pconst.tile(shape, dt, tag=tag)
                nc.vector.tensor_copy(t[:], dma_t[:])
                return t

            bias_e_t = dve_const(bias_e[:], [128, NG], f32, "be")
            bias_d_t = dve_const(bias_d[:], [128, NG], f32, "bd")
            ones_t = pconst.tile([128, 1], f32)
            nc.vector.memset(ones_t[:], 1.0)
            ident = pconst.tile([128, 128], bf16)
            nc.sync.dma_start(ident[:], ident_in[:])

            # HT: decoder hidden states, transposed, col = k*SBP + t*64 + b
            ht = pht.tile([128, KC * SBP], bf16)
            nc.vector.memset(ht[:], 0.0)

            def load_w(dram, pool, tag, width=4 * H):
                ts = []
                dw = dram.shape[2]
                for k in range(KC):
                    t = pool.tile([128, width], bf16, tag=f"{tag}{k}")
                    nc.sync.dma_start(t[:, :dw], dram[k])
                    ts.append(t)
                return ts

            we_i = load_w(wi_e, pw, "wie")
            we_h = load_w(wh_e, pw, "whe")
            wd_i = load_w(wi_d, pw, "wid")
            wd_h = load_w(wh_d, pw, "whd")

            # ============ unified 95-step recurrence ============
            with (
                tc.tile_pool(name="psA", bufs=3, space=bass.MemorySpace.PSUM)
                    as psA,
                tc.tile_pool(name="psB", bufs=2, space=bass.MemorySpace.PSUM)
                    as psB,
                tc.tile_pool(name="psC", bufs=2, space=bass.MemorySpace.PSUM)
                    as psC,
            ):
                def bulk_pieces(xt_src, wi_t, bias_t, t0, nsteps):
                    """Yield closures: piece 0 = DMA + gx alloc, one piece
                    per gate chunk (4 MMs + bias copy to gx), then a
                    sentinel returning the gx tile."""
                    w = nsteps * B
                    state = {}

                    def p_dma():
                        state["gx"] = pgx.tile([128, NG * WIN * B], bf16,
                                               tag="gx", name="gxw")
                        xtw = []
                        for k in range(KC):
                            t = pxt.tile([128, WIN * B], bf16, tag=f"xt{k}")
                            nc.sync.dma_start(
                                t[:, :w], xt_src[k * 128:(k + 1) * 128,
                                                 t0 * B:t0 * B + w])
                            xtw.append(t)
                        state["xtw"] = xtw
                    yield p_dma

                    def mk_chunk(g):
                        def p_chunk():
                            pb = psA.tile([128, 512], f32, tag="psA")
                            for k in range(KC):
                                nc.tensor.matmul(
                                    pb[:, :w],
                                    wi_t[k][:, g * 128:(g + 1) * 128],
                                    state["xtw"][k][:, :w],
                                    start=(k == 0), stop=(k == KC - 1))
                            nc.vector.tensor_scalar_add(
                                state["gx"][:, g * WIN * B:g * WIN * B + w],
                                pb[:, :w], bias_t[:, g:g + 1])
                        return p_chunk
                    for g in range(NG):
                        yield mk_chunk(g)
                    yield lambda: state["gx"]

                def lstm_step(gx, lt, h_rhs, c_prev, wh_t, h_out_ap):
                    """One step. g chunks issue first (c-path overlaps the
                    i/f/o matmuls), o last (short h tail)."""
                    pA = psA.tile([128, 512], f32, tag="psA")  # i|f
                    pB = psB.tile([128, 256], f32, tag="psB")  # g
                    pC = psC.tile([128, 256], f32, tag="psC")  # o

                    def dst(c):
                        if c in G_CH:
                            return pB[:, (c - 12) * B:(c - 11) * B]
                        if c in O_CH:
                            return pC[:, (c - 8) * B:(c - 7) * B]
                        return pA[:, c * B:(c + 1) * B]
                    order = G_CH + I_CH + F_CH + O_CH
                    gx_r = gx[:].rearrange("p (g s) -> p g s", g=NG)
                    # x-part injection: identity stationary, one wide
                    # matmul per PSUM tile (one accumulation group each)
                    nc.tensor.matmul(
                        pB[:].rearrange("p (g s) -> p g s", g=4),
                        ident[:], gx_r[:, 12:16, lt * B:(lt + 1) * B],
                        start=True, stop=False)
                    nc.tensor.matmul(
                        pA[:].rearrange("p (g s) -> p g s", g=8),
                        ident[:], gx_r[:, 0:8, lt * B:(lt + 1) * B],
                        start=True, stop=False)
                    nc.tensor.matmul(
                        pC[:].rearrange("p (g s) -> p g s", g=4),
                        ident[:], gx_r[:, 8:12, lt * B:(lt + 1) * B],
                        start=True, stop=False)
                    # h-part; last matmul into each tile carries stop
                    for c in order:
                        for k in range(KC):
                            last = (k == KC - 1) and c in (15, 7, 11)
                            nc.tensor.matmul(
                                dst(c),
                                wh_t[k][:, c * 128:(c + 1) * 128],
                                h_rhs(k),
                                start=False, stop=last)
                    # ACT: g's tanh first (its matmuls finished first)
                    tng = pact.tile([128, 256], f32, tag="tng")
                    nc.scalar.activation(tng[:], pB[:], AF.Tanh)
                    sig = pact.tile([128, 512], f32, tag="sig")
                    nc.scalar.activation(sig[:], pA[:], AF.Sigmoid)
                    sgo = pact.tile([128, 256], f32, tag="sgo")
                    nc.scalar.activation(sgo[:], pC[:], AF.Sigmoid)
                    # c2 = sig_f*c + sig_i*tanh_g
                    t2 = pact.tile([128, 256], f32, tag="t2")
                    nc.vector.tensor_mul(t2[:], sig[:, 0:256], tng[:])
                    t1 = pact.tile([128, 256], f32, tag="t1")
                    nc.vector.tensor_mul(t1[:], sig[:, 256:512], c_prev[:])
                    c_new = pstate.tile([128, 256], f32, tag="c")
                    nc.vector.tensor_add(c_new[:], t1[:], t2[:])
                    tnc = pact.tile([128, 256], f32, tag="tnc")
                    nc.scalar.activation(tnc[:], c_new[:], AF.Tanh)
                    nc.vector.tensor_mul(
                        h_out_ap,
                        sgo[:].rearrange("p (k s) -> p k s", k=KC),
                        tnc[:].rearrange("p (k s) -> p k s", k=KC))
                    return c_new

                h_prev = pstate.tile([128, KC * B], bf16, tag="h")
                nc.vector.memset(h_prev[:], 0.0)
                c_prev = pstate.tile([128, 256], f32, tag="c")
                nc.vector.memset(c_prev[:], 0.0)

                win_list = (
                    [(xt_enc, we_i, bias_e_t, t0, min(WIN, SRC - t0))
                     for t0 in range(0, SRC, WIN)] +
                    [(xt_dec, wd_i, bias_d_t, t0, min(WIN, DEC - t0))
                     for t0 in range(0, DEC, WIN)])

                gx = None
                for p in bulk_pieces(*win_list[0]):   # prologue window
                    r = p()
                    gx = r if r is not None else gx
                next_idx = 1
                next_gen = bulk_pieces(*win_list[next_idx])
                gx_next = None

                step_no = 0
                for phase, nsteps in (("enc", SRC), ("dec", DEC)):
                    wh_t = we_h if phase == "enc" else wd_h
                    for t in range(nsteps):
                        if t % WIN == 0 and step_no > 0:
                            # window switch: finish pending bulk, swap gx
                            while next_gen is not None:
                                try:
                                    p = next(next_gen)
                                except StopIteration:
                                    next_gen = None
                                    break
                                r = p()
                                gx_next = r if r is not None else gx_next
                            gx, gx_next = gx_next, None
                            next_idx += 1
                            if next_idx < len(win_list):
                                next_gen = bulk_pieces(*win_list[next_idx])
                        if phase == "enc" or t == 0:
                            hp = h_prev
                            rhs = (lambda k, hp=hp:
                                   hp[:, k * B:(k + 1) * B])
                        else:
                            rhs = (lambda k, tp=t - 1:
                                   ht[:, k * SBP + tp * B:
                                      k * SBP + (tp + 1) * B])
                        if phase == "enc":
                            h_new = pstate.tile([128, KC * B], bf16, tag="h")
                            out_ap = h_new[:].rearrange(
                                "p (k s) -> p k s", k=KC)
                        else:
                            out_ap = ht[:].rearrange(
                                "p (k s) -> p k s",
                                k=KC)[:, :, t * B:(t + 1) * B]
                        c_new = lstm_step(gx, t % WIN, rhs, c_prev, wh_t,
                                          out_ap)
                        if phase == "enc":
                            mk = pmask.tile([128, KC * B], mybir.dt.uint8,
                                            tag="mk")
                            nc.sync.dma_start(mk[:], mask_in[t])
                            nc.vector.copy_predicated(h_new[:], mk[:],
                                                      h_prev[:])
                            nc.vector.copy_predicated(c_new[:], mk[:],
                                                      c_prev[:])
                            h_prev = h_new
                        c_prev = c_new
                        step_no += 1
                        # interleave next window's bulk (2 pieces/step)
                        if next_gen is not None:
                            for _ in range(2):
                                try:
                                    p = next(next_gen)
                                except StopIteration:
                                    next_gen = None
                                    break
                                r = p()
                                gx_next = r if r is not None else gx_next

            # ================= target logits =================
            with tc.tile_pool(name="psL", bufs=2,
                              space=bass.MemorySpace.PSUM) as psL:
                def load_w2(dram, nm):
                    # two [128, 2*VSP] tiles in the (now idle) gx slots
                    dw = dram.shape[2]
                    ts = []
                    for half in range(2):
                        t = pgx.tile([128, 2 * VSP], bf16, tag="gx",
                                     name=f"{nm}{half}")
                        for j in range(2):
                            nc.sync.dma_start(
                                t[:, j * VSP:j * VSP + dw], dram[half * 2 + j])
                        ts.append(t)
                    return lambda k: ts[k // 2][:, (k % 2) * VSP:
                                                (k % 2 + 1) * VSP]

                wtg_s = load_w2(wtgt_in, "wtg")
                l_sb = pconst.tile([1, SB], f32)
                for nt in range(6):
                    wdt = min(512, SB - nt * 512)
                    pt = psL.tile([128, 2048], f32, tag="psL")
                    for k in range(KC):
                        prod = plog.tile([128, 512], f32, tag="prod")
                        nc.vector.tensor_mul(
                            prod[:, :wdt],
                            ht[:, k * SBP + nt * 512:k * SBP + nt * 512 + wdt],
                            wtg_s(k)[:, nt * 512:nt * 512 + wdt])
                        nc.tensor.matmul(pt[0:1, :wdt], ones_t[:],
                                         prod[:, :wdt],
                                         start=(k == 0), stop=(k == KC - 1))
                    nc.scalar.copy(l_sb[:, nt * 512:nt * 512 + wdt],
                                   pt[0:1, :wdt])
                nc.sync.dma_start(out_l[:], l_sb[:])

                # ============ vocab-shard logits + sum-exp ============
                wot_s = load_w2(wot_in, "wot")
                bout = pconst.tile([128, VSP], bf16)
                nc.sync.dma_start(bout[:], bout_in[:])
                s_all = pconst.tile([128, SBC], f32)

                for sb in range(SBC):
                    sh = []
                    for half in range(2):
                        pl = psL.tile([128, 2048], f32, tag="psL")
                        for v4 in range(4):
                            col = half * 2048 + v4 * 512
                            for k in range(KC):
                                nc.tensor.matmul(
                                    pl[:, v4 * 512:(v4 + 1) * 512],
                                    ht[:, k * SBP + sb * 128:
                                       k * SBP + (sb + 1) * 128],
                                    wot_s(k)[:, col:col + 512],
                                    start=(k == 0), stop=(k == KC - 1))
                        for q in range(2):
                            qs = slice(q * 1024, (q + 1) * 1024)
                            nc.vector.tensor_add(
                                pl[:, qs], pl[:, qs],
                                bout[:, half * 2048 + q * 1024:
                                     half * 2048 + (q + 1) * 1024])
                            sh_t = plog.tile([128, 1], f32,
                                             tag=f"sh{half * 2 + q}",
                                             name="sh_t")
                            nc.scalar.activation(pl[:, qs], pl[:, qs],
                                                 AF.Exp, accum_out=sh_t[:])
                            sh.append(sh_t)
                    sh01 = plog.tile([128, 1], f32, tag="sh01")
                    nc.vector.tensor_add(sh01[:], sh[0][:], sh[1][:])
                    sh23 = plog.tile([128, 1], f32, tag="sh23")
                    nc.vector.tensor_add(sh23[:], sh[2][:], sh[3][:])
                    nc.vector.tensor_add(s_all[:, sb:sb + 1],
                                         sh01[:], sh23[:])
                nc.sync.dma_start(out_s[:], s_all[:])

    nc.compile()
    return nc


def _prep(inputs):
    """Host-side data prep. Returns per-core in_maps + host combine data."""
    il = np.asarray(inputs["input_lines"])
    tl = np.asarray(inputs["target_lines"])
    f = lambda k: np.asarray(inputs[k], np.float32)
    emb_in, emb_tgt = f("emb_in").copy(), f("emb_tgt").copy()
    emb_in[0] = 0.0
    emb_tgt[0] = 0.0
    W_out, b_out = f("W_out"), f("b_out")

    perm = np.concatenate([np.arange(0, 512), np.arange(512, 1024),
                           np.arange(1536, 2048), np.arange(1024, 1536)])

    def wt(w):  # [2048,512] -> [4,128,2048] bf16 (transposed, gate-permuted)
        return np.ascontiguousarray(
            w[perm].T.reshape(KC, 128, 4 * H)).astype(BF16)

    def bias(bi, bh):  # -> [128, 16] f32
        return np.ascontiguousarray(
            (bi + bh)[perm].reshape(NG, 128).T).astype(np.float32)

    x_enc = emb_in[il.reshape(-1)]                       # [3072, 512]
    xt_enc = np.ascontiguousarray(x_enc.T).astype(BF16)  # [512, 3072]
    tgt_in = tl[:DEC].reshape(-1)
    x_dec = emb_tgt[tgt_in]
    xt_dec = np.ascontiguousarray(x_dec.T).astype(BF16)  # [512, 3008]

    m = (il == 0).astype(np.uint8)                       # [48, 64]
    mask = np.ascontiguousarray(np.broadcast_to(
        m[:, None, None, :], (SRC, 128, KC, B)).reshape(
            SRC, 128, KC * B)).astype(np.uint8)

    tgt_next = tl[1:TGT].reshape(-1)                     # [3008]
    wtgt = np.ascontiguousarray(
        W_out[tgt_next].T.reshape(KC, 128, SB)).astype(BF16)
    b_tgt = b_out[tgt_next].astype(np.float64)

    common = dict(
        xt_enc=xt_enc, xt_dec=xt_dec,
        wi_e=wt(f("W_ih_e")), wh_e=wt(f("W_hh_e")),
        wi_d=wt(f("W_ih_d")), wh_d=wt(f("W_hh_d")),
        bias_e=bias(f("b_ih_e"), f("b_hh_e")),
        bias_d=bias(f("b_ih_d"), f("b_hh_d")),
        mask=mask, wtgt=wtgt,
        ident=np.eye(128, dtype=BF16),
    )
    in_maps = []
    for c in range(NCORES):
        ws = np.zeros((VSP, H), np.float32)
        ws[:VSH] = W_out[c * VSH:(c + 1) * VSH]
        bs = np.full(VSP, -88.0, np.float32)
        bs[:VSH] = b_out[c * VSH:(c + 1) * VSH]
        in_maps.append(dict(
            common,
            wot=np.ascontiguousarray(ws.T.reshape(KC, 128, VSP)).astype(BF16),
            bout=np.ascontiguousarray(
                np.broadcast_to(bs, (128, VSP))).astype(BF16),
        ))
    return in_maps, b_tgt


def _combine(results, b_tgt):
    s = np.zeros(SBP, np.float64)
    for r in results:
        s += np.asarray(r["out_s"], np.float64).T.reshape(-1)
    s = s[:SB]
    lse = np.log(s)
    l_tgt = np.asarray(results[0]["out_l"], np.float64).reshape(-1) + b_tgt
    return np.float32((lse - l_tgt).sum() / B)


def kernel(**inputs):
    global _COMPILED
    from concourse.bass_utils import run_bass_kernel_spmd
    in_maps, b_tgt = _prep(inputs)
    if _COMPILED is None:
        _COMPILED = _build()
    res = run_bass_kernel_spmd(_COMPILED, in_maps, list(range(NCORES)))
    return _combine(res.results, b_tgt)


if __name__ == "__main__":
    import reference
    inp = reference.setup_inputs()
    expected = np.asarray(reference.reference(**inp))
    actual = kernel(**{k: np.asarray(v) for k, v in inp.items()})
    err = abs(actual - expected) / max(abs(expected), 1e-9)
    print(f"expected={expected} actual={actual} rel_err={err:.3e}")
